# revision 33
# baseline (speedup 1.0000x reference)
"""Trainium2 Bass kernel for nn_Encoder_21964462752332.

Math: the swap-test circuit per 4x4 patch p reduces to
    out = (num + den) / (2 den),  num = ||A p||^2, den = ||p||^2,
with A = U[:4, :] the first 4 rows of the 16x16 orthogonal MPS circuit
matrix built on host from the 12 weights_mps floats.

Device algorithm (per core, 32 images, space-to-depth layout, bf16):
  I4[(h,w,c2), b, r2] = I[b, 2*r2+h, 2*c2+w]   (host-packed, [128, b, 32])
  y[(j,ow), b, oh]    = sum_k  W1k^T @ I4[:, :, oh+k]     2 accumulating
                        matmuls; vertical shift via rhs free-dim views,
                        horizontal taps via banded lhsT weights
  ysq = y^2 (ACT, PSUM->SBUF bf16); Isq = I4^2, v = Isq[..0:31]+Isq[..1:32]
  z   = wsd^T @ ysq + wv^T @ v  -> z[0:31]=num+den, z[32:63]=2*den
  out = z[0:31] * recip(z[32:63])   (f32 for accuracy; bf16 elsewhere)

All weight matrices are padded to 32-col slots with output channel
m = j*32 + ow so the whole input (weights + 32 images) is one HBM blob
loaded by three contiguous DMAs ([wts+chunk0] first so compute starts
as early as possible).  Dummy warmup matmuls keep the PE p-state
ramping during the DMA fill (matmuls then run at 2.4GHz instead of
0.65/1.2GHz).  Output leaves via two direct kv_writeback DMAs (one per
16-image half) so the first half's writeback overlaps the second
half's compute.  vs the im2col baseline (22476 ns modeled): 4.4x less
HBM traffic, ~2x fewer PE columns -> 11853 ns modeled, rel err 3.1e-3.

Measured on trn2 via run_bass_kernel_spmd: rel err 3.125e-03.
NOTE: the SWDGE prepare_only + trigger_dma path (which would cut
another ~2us of tail latency per TimelineSim) compiles and simulates
but corrupts output / crashes the device on this HW/toolchain
(NRT_EXEC_UNIT_UNRECOVERABLE); wb_direct=True is the stable config.
"""

import numpy as np

# ---- problem geometry (hardcoded per contract) ----
BS = 256
H = W = 64
OH = OW = 31
N_CORES = 8
NB = BS // N_CORES              # 32 images per core
NCHUNK = 2                      # default b-chunks per core (cfg "nchunk")

WSLOTS = 12                     # weight slots of 32 cols: w1_0(4) w1_1(4) wsd(2) wv(2)
BSLOTS = WSLOTS + NB            # + image slots

_CACHE = {}
TRACE = False
TRACE_KWARGS = {}

CFG = dict(
    nchunk=4,          # b-chunks per core
    warmup=2,          # PE p-state warmup matmuls
    warmup_cols=512,   # rhs width of each warmup matmul
    warmup_memset=True,  # zero the dummy tile first (False: garbage matmuls)
    tail_pool=False,   # odd chunks' final mult on GPSIMD (parallel with DVE)
    out_split=False,   # separate output DMA per chunk
    fused_isq=False,   # whole-image Isq/v in single DVE ops
    out_wb=True,       # output via kv_writeback (direct)
    wb_direct=True,    # kv_writeback fired immediately (no prep/trigger)
    wb_bridge=True,    # sim-sem bridge for the deferred path
    wb_q0=False,       # deferred preps all on queue 0, count=1 triggers
    tail_bf16=False,   # zs/recip/mult/res/writeback in bf16 (host casts to f32)
)


def _build_U(weights_mps: np.ndarray) -> np.ndarray:
    """16x16 orthogonal MPS circuit matrix; amp index bits are MSB-first in
    local data-wire order (wire 0 = most significant)."""
    Wm = np.asarray(weights_mps, dtype=np.float64)
    I2 = np.eye(2)
    CNOT = np.array(
        [[1, 0, 0, 0], [0, 1, 0, 0], [0, 0, 0, 1], [0, 0, 1, 0]], dtype=np.float64
    )

    def ry(t):
        c, s = np.cos(t / 2.0), np.sin(t / 2.0)
        return np.array([[c, -s], [s, c]])

    def emb1(U2, w):
        out = np.array([[1.0]])
        for i in range(4):
            out = np.kron(out, U2 if i == w else I2)
        return out

    def emb2(U4, w):
        return np.kron(np.eye(2 ** w), np.kron(U4, np.eye(2 ** (2 - w))))

    U = np.eye(16)
    for l in range(2):
        for b in range(3):
            U = emb1(ry(Wm[l, b, 0]), b) @ U
            U = emb1(ry(Wm[l, b, 1]), b + 1) @ U
            U = emb2(CNOT, b) @ U
    return U


def _build_device_weights(U: np.ndarray) -> np.ndarray:
    """[128, WSLOTS*32] f32; output channel m = j*32 + ow (ow 0..30)."""
    A = U[:4, :]  # [4, 16] over taps t = kh*4 + kw
    wts = np.zeros((128, WSLOTS * 32), dtype=np.float32)
    for kap in range(2):
        w1 = wts[:, kap * 128:(kap + 1) * 128]
        for h in range(2):
            for w in range(2):
                for j in range(4):
                    for ow in range(OW):
                        for d in (0, 1):
                            p = h * 64 + w * 32 + ow + d
                            w1[p, j * 32 + ow] = A[j, (2 * kap + h) * 4 + 2 * d + w]
    wsd = wts[:, 256:320]
    for j in range(4):
        for ow in range(OW):
            wsd[j * 32 + ow, ow] = 1.0
    wv = wts[:, 320:384]
    for h in range(2):
        for w in range(2):
            for ow in range(OW):
                for d in (0, 1):
                    p = h * 64 + w * 32 + ow + d
                    wv[p, ow] = 1.0
                    wv[p, 32 + ow] = 2.0
    return wts


def _build_bass(loop_reps=None, loop_unroll=1, cfg=None):
    import concourse.bacc as bacc
    import concourse.mybir as mybir
    from concourse.tile import TileContext

    cfg = dict(CFG, **(cfg or {}))
    nchunk = cfg["nchunk"]
    carb = NB // nchunk            # images per chunk
    free = carb * OH               # moving columns per matmul (<=512)
    f32 = mybir.dt.float32
    bf16 = mybir.dt.bfloat16
    nc = bacc.Bacc(None, num_swdge_queues=3)

    blob_d = nc.dram_tensor("blob", [128, BSLOTS * 32], bf16, kind="ExternalInput")
    tdt = bf16 if cfg["tail_bf16"] else f32
    if cfg["out_wb"]:
        out_d = nc.dram_tensor("out", [2, 1, 128, 1, 512], tdt, kind="ExternalOutput")
    else:
        out_d = nc.dram_tensor("out", [OW, NB * OH], tdt, kind="ExternalOutput")

    with TileContext(nc) as tc:
        with (
            tc.tile_pool(name="big", bufs=1) as bigpool,
            tc.tile_pool(name="work", bufs=1) as wpool,
            tc.tile_pool(name="psumw", bufs=1, space="PSUM") as ppoolw,
            tc.tile_pool(name="psumy", bufs=2, space="PSUM") as ppooly,
            tc.tile_pool(name="psumz", bufs=2, space="PSUM") as ppoolz,
        ):
            big = bigpool.tile([128, BSLOTS, 32], bf16)
            if cfg["fused_isq"]:
                isqall = bigpool.tile([128, NB, 32], bf16, tag="isqall")
                vall = bigpool.tile([128, NB, OH], bf16, tag="vall")
                isqs = None
                vs = [vall[:, c * (NB // cfg["nchunk"]):(c + 1) * (NB // cfg["nchunk"]), :]
                      for c in range(cfg["nchunk"])]
            else:
                isqs = [bigpool.tile([128, carb, 32], bf16, name=f"isq{c}", tag=f"isq{c}")
                        for c in range(nchunk)]
                vs = [bigpool.tile([128, carb, OH], bf16, name=f"v{c}", tag=f"v{c}")
                      for c in range(nchunk)]
            if cfg["out_wb"]:
                ress = [bigpool.tile([128, 1, 1, 512], tdt, name=f"res{h}",
                                     tag=f"res{h}") for h in range(2)]
            else:
                res1 = bigpool.tile([OW, NB, OH], tdt, tag="res1")
            wuc = cfg["warmup_cols"]
            dummy = bigpool.tile([128, 64 + wuc], bf16, tag="dummy")

            w1 = [big[:, 0:4, :], big[:, 4:8, :]]
            wsd = big[:, 8:10, :]
            wv = big[:, 10:12, :]
            imgs = [big[:, WSLOTS + c * carb:WSLOTS + (c + 1) * carb, :]
                    for c in range(nchunk)]
            if cfg["out_wb"] and not cfg["wb_direct"]:
                sem_o = [nc.alloc_semaphore(name=f"odma{h}") for h in range(2)]
                psems = [nc.alloc_semaphore(name=f"oprep{h}") for h in range(2)]
            else:
                sem_o = psems = None

            def body():
                # ---- input loads: [weights+chunk0], [chunk1], [rest] ----
                cuts = [0, WSLOTS + carb]
                if nchunk > 1:
                    cuts.append(WSLOTS + 2 * carb)
                cuts.append(BSLOTS)
                for a, b in zip(cuts[:-1], cuts[1:]):
                    if a < b:
                        nc.sync.dma_start(
                            out=big[:, a:b, :], in_=blob_d[:, a * 32:b * 32],
                        )

                # ---- output writeback preps (desc-gen during the fill) ----
                if cfg["out_wb"]:
                    cidx = wpool.tile([128, 1], mybir.dt.int32, tag="cidx")
                    nc.gpsimd.iota(cidx[:], pattern=[[0, 1]], base=0,
                                   channel_multiplier=0)
                    if not cfg["wb_direct"]:
                        if cfg["wb_q0"]:
                            nc.gpsimd.kv_writeback(
                                out_d[0], ress[0][:], cidx[:],
                                prepare_only=True, sem=sem_o[0], queue_num=0,
                            ).then_inc(psems[0], 1)
                        else:
                            for h in range(2):
                                nc.gpsimd.kv_writeback(
                                    out_d[h], ress[h][:], cidx[:],
                                    prepare_only=True, sem=sem_o[h],
                                    queue_num=h,
                                )

                # ---- PE p-state warmup on dummy data ----
                if cfg["warmup"]:
                    if cfg["warmup_memset"]:
                        nc.gpsimd.memset(dummy[:], 0.0)
                    wps = ppoolw.tile([64, wuc], f32, tag="warm")
                    for _ in range(cfg["warmup"]):
                        nc.tensor.matmul(
                            wps[:], lhsT=dummy[:, 0:64], rhs=dummy[:, 64:64 + wuc],
                            start=True, stop=True,
                        )

                # ---- per-chunk compute ----
                if cfg["fused_isq"]:
                    imgall = big[:, WSLOTS:BSLOTS, :]
                    nc.vector.tensor_tensor(
                        isqall[:], imgall[:], imgall[:], mybir.AluOpType.mult,
                    )
                    nc.vector.tensor_tensor(
                        vall[:], isqall[:, :, 0:OH], isqall[:, :, 1:OH + 1],
                        mybir.AluOpType.add,
                    )
                zss = []
                for c in range(nchunk):
                    img = imgs[c]
                    yp = ppooly.tile([128, free], f32, name=f"yp{c}", tag="y")
                    nc.tensor.matmul(
                        yp[:], lhsT=w1[0], rhs=img[:, :, 0:OH],
                        start=True, stop=False,
                    )
                    nc.tensor.matmul(
                        yp[:], lhsT=w1[1], rhs=img[:, :, 1:OH + 1],
                        start=False, stop=True,
                    )

                    ysq = wpool.tile([128, free], bf16, name=f"ysq{c}", tag=f"ysq{c}")
                    nc.scalar.activation(
                        ysq[:], yp[:], mybir.ActivationFunctionType.Square
                    )

                    if not cfg["fused_isq"]:
                        nc.vector.tensor_tensor(
                            isqs[c][:], img[:], img[:], mybir.AluOpType.mult,
                        )
                        nc.vector.tensor_tensor(
                            vs[c][:], isqs[c][:, :, 0:OH],
                            isqs[c][:, :, 1:OH + 1], mybir.AluOpType.add,
                        )

                    zp = ppoolz.tile([64, carb, OH], f32, name=f"zp{c}", tag="z")
                    nc.tensor.matmul(
                        zp[:], lhsT=wsd, rhs=ysq[:],
                        start=True, stop=False, skip_group_check=True,
                    )
                    vr = vs[c] if cfg["fused_isq"] else vs[c][:]
                    nc.tensor.matmul(
                        zp[:], lhsT=wv, rhs=vr,
                        start=False, stop=True, skip_group_check=True,
                    )

                    zs = wpool.tile([64, carb, OH], tdt, name=f"zs{c}", tag=f"zs{c}")
                    nc.scalar.copy(zs[:], zp[:])
                    zss.append(zs)

                import contextlib
                lp = (nc.allow_low_precision(reason="bf16 normalize; validated 2e-2 gate")
                      if cfg["tail_bf16"] else contextlib.nullcontext())
                with lp:
                 for c in range(nchunk):
                    eng = (nc.gpsimd if (cfg["tail_pool"] and c % 2 == 1)
                           else nc.vector)
                    rden = wpool.tile([OW, carb, OH], tdt, name=f"rden{c}",
                                      tag=f"rden{c}")
                    nc.vector.reciprocal(rden[:], zss[c][32:63, :, :])
                    if cfg["out_wb"]:
                        # halves: chunks [0, nchunk/2) -> res0, rest -> res1
                        half = (2 * c) // nchunk
                        o = (c - half * (nchunk // 2)) * free
                        eng.tensor_tensor(
                            ress[half][0:OW, 0, 0, o:o + free],
                            zss[c][0:OW, :, :], rden[:], mybir.AluOpType.mult,
                        )
                        if (c + 1) % (nchunk // 2) == 0:
                            if cfg["wb_direct"]:
                                nc.gpsimd.kv_writeback(
                                    out_d[half], ress[half][:], cidx[:],
                                    queue_num=half,
                                )
                            elif cfg["wb_q0"]:
                                nc.gpsimd.wait_ge(psems[half], 1)
                                nc.gpsimd.trigger_dma(count=1, queue_num=0)
                                if half == 0:
                                    nc.gpsimd.kv_writeback(
                                        out_d[1], ress[1][:], cidx[:],
                                        prepare_only=True, sem=sem_o[1],
                                        queue_num=0,
                                    ).then_inc(psems[1], 1)
                            else:
                                nc.gpsimd.trigger_dma(count=None,
                                                      queue_num=half)
                        continue
                    eng.tensor_tensor(
                        res1[:, c * carb:(c + 1) * carb, :], zss[c][0:OW, :, :],
                        rden[:], mybir.AluOpType.mult,
                    )
                    if cfg["out_split"]:
                        nc.gpsimd.dma_start(
                            out=out_d[:, c * carb * OH:(c + 1) * carb * OH],
                            in_=res1[:, c * carb:(c + 1) * carb, :],
                        )
                if cfg["out_wb"]:
                    # sim/HW sem bridge: TimelineSim fires only the baked
                    # fresh sem; Tile's epilogue waits its DMASW lane sems
                    # (auto-bumped by the ring on HW).  wait+inc satisfies
                    # the sim; on HW both fire at/after DMA completion.
                    if not cfg["wb_direct"] and cfg["wb_bridge"]:
                        for h in range(2):
                            nc.gpsimd.wait_ge(sem_o[h], 16)
                            nc.gpsimd.sem_inc(
                                tc.sems.swdge_block()[0 if cfg["wb_q0"] else h],
                                16)
                elif not cfg["out_split"]:
                    nc.gpsimd.dma_start(out=out_d[:], in_=res1[:])

            if loop_reps is None:
                body()
            else:
                with tc.For_i(0, loop_reps, 1):
                    for _ in range(loop_unroll):
                        body()
    nc.compile()
    return nc


def _get_bass():
    if "nc" not in _CACHE:
        _CACHE["nc"] = _build_bass()
    return _CACHE["nc"]


def _prep_inputs(img, weights_mps):
    import ml_dtypes

    bf16 = ml_dtypes.bfloat16
    img = np.asarray(img, dtype=np.float32)[:, 0]  # [256, 64, 64]
    U = _build_U(weights_mps)
    wts = _build_device_weights(U)

    # space-to-depth: I4[core, (h,w,c2), b, r2] = I[core*NB+b, 2*r2+h, 2*c2+w]
    I = img.reshape(N_CORES, NB, H, W)
    I4 = np.empty((N_CORES, 128, NB, 32), dtype=np.float32)
    for h in range(2):
        for w in range(2):
            blk = I[:, :, h::2, w::2]           # [cores, b, r2, c2]
            I4[:, h * 64 + w * 32:h * 64 + w * 32 + 32] = blk.transpose(0, 3, 1, 2)
    blobs = np.concatenate(
        [np.broadcast_to(wts[None], (N_CORES,) + wts.shape),
         I4.reshape(N_CORES, 128, NB * 32)], axis=2
    ).astype(bf16)
    return np.ascontiguousarray(blobs)


def kernel(img: np.ndarray, weights_mps: np.ndarray) -> np.ndarray:
    from concourse.bass_utils import run_bass_kernel_spmd

    blobs = _prep_inputs(img, weights_mps)
    nc = _get_bass()
    in_maps = [{"blob": blobs[c]} for c in range(N_CORES)]
    r = run_bass_kernel_spmd(
        nc, in_maps, list(range(N_CORES)), trace=TRACE, **TRACE_KWARGS
    )
    if TRACE:
        _CACHE["last_result"] = r

    outs = np.stack([r.results[c]["out"] for c in range(N_CORES)])
    if CFG["out_wb"]:
        # [cores, 2, 1, 128, 1, 512] -> halves of 16 images, (ow, b, oh)
        outs = outs[:, :, 0, 0:OW, 0, 0:16 * OH].reshape(N_CORES, 2, OW, 16, OH)
        outs = outs.transpose(0, 1, 3, 4, 2)   # [cores, half, b, oh, ow]
    else:
        # [cores, OW, NB*OH] -> (b, oh, ow)
        outs = outs.reshape(N_CORES, OW, NB, OH).transpose(0, 2, 3, 1)
    return np.ascontiguousarray(
        outs.reshape(BS, 1, OH * OW).astype(np.float32)
    )
